# revision 26
# baseline (speedup 1.0000x reference)
"""Trainium2 Bass kernel for a Chemprop GNN message-passing layer.

Reference computation (single layer, n_nodes=50000, n_edges=300000, hidden=256):
    H   = relu(E)                                  # [E, 256]
    M_v = segment_sum(H, dest, n_nodes)            # [V, 256]
    out = (M_v[src] - H[rev]) @ W.T + b            # [E, 256]

Distribution over 8 NeuronCores (zero collectives): nodes are sharded; core c
owns node range [c*6250, (c+1)*6250), as 49 blocks of 128 node lanes.

Host prep (pure permutation / gather / relu, all O(E*H)):
  * Phase 1 (segment sum): edges grouped by dest-node block; relu(E) rows
    written to a [128, NBLK*CPB1*256] f16 slab in (lane, block, chunk, hid)
    order so each block's DMA is contiguous per partition.
  * Phase 2: edges grouped by src-node block. relu(E)[rev] rows are
    PRE-GATHERED on host and stored TRANSPOSED per chunk:
    ERV[p, (blk*CPB2+j)*256 + r*128 + i] = relu(E[rev[slot(blk,j,i)]])[r*128+p].
    This removes all device-side indirect gathers and PE transposes.

Device per block bb (software-pipelined: phase1(bb+1) emitted before
phase2(bb) so PE never stalls on the Mv PSUM->SBUF copy):
  phase 1: stream h_blk; per chunk build one-hot S[e,n]=(dest_lane[e]==n) on
    DVE and accumulate Mv_ps += S.T @ H_chunk on PE; ScalarE copies Mv to a
    resident f16 SBUF tile (49 x [128,256]).
  phase 2: stream erv (already relu'd, transposed, negatable); build
    R[n,e]=(src_lane[e]==n) from a partition_broadcast (GPSIMD) of the src
    lane row; per chunk:
      pv_T[r*128+q, i] = Mv[:, r-half].T @ R  (2 matmuls into one PSUM tile)
      muv_T = pv_T - erv_chunk                (DVE, f16 out)
      out_ps = muv_T[0:128].T @ W.T[0:128] + muv_T[128:256].T @ W.T[128:256]
      ScalarE copies out_ps (f32) -> out_blk (f16)
    One DMA writes the block's outputs; bias is added on host in assemble().
"""

import sys
from contextlib import ExitStack

import numpy as np

sys.path.insert(0, "/opt/trn_rl_repo")

import concourse.bass as bass
import concourse.bacc as bacc
import concourse.tile as tile
from concourse import mybir
from concourse.bass_utils import run_bass_kernel_spmd

N_NODES = 50000
N_EDGES = 300000
HID = 256
NC = 8
P = 128
NPC = N_NODES // NC          # 6250 nodes per core
NBLK = (NPC + P - 1) // P    # 49 blocks of 128 node lanes per core
PAD_LANE = 200.0             # sentinel lane value -> one-hot row of zeros
SB_BUFS = 4
PS_BUFS = (2, 4)             # psum bufs: mv, out
OUT_COPY_ENG = "scalar"      # engine for PSUM->SBUF out copies
MV_COPY_ENG = "vector"       # engine for PSUM->SBUF Mv copies
OUT_DMA_ALT = False          # alternate out-write queue between SP/Act
H_ENG = "scalar"             # queue for h_blk loads
ERV_ENG = "sync"             # queue for erv loads
OUT_ENG = "sync"             # queue for out writes (when not alternating)
SPLIT_MUV = False            # subtract muv halves separately
SKIP_P1MM = False            # ablation: skip phase-1 s_t + matmuls
SKIP_LIN = False             # ablation: skip linear matmuls + out copies
SKIP_GATHER = False          # ablation: skip gather+subtract (muv <- erv)
QUEUE_RR = False             # round-robin h/erv/out across both HWDGE queues


def _groups(cpb):
    """Split cpb chunks into pairs (last group may be a single)."""
    gs = []
    j = 0
    while j < cpb:
        w = min(2, cpb - j)
        gs.append((j, w))
        j += w
    return gs


def _group_slots(node_ids):
    """Group edges by (core, block) of node ownership; assign (chunk, lane)
    slots. Returns (order, core, blk, j, p, lane, CPB)."""
    c = node_ids // NPC
    loc = node_ids - c * NPC
    blk = loc >> 7
    lane = loc & 127
    g = c * NBLK + blk
    order = np.argsort(g, kind="stable")
    gs = g[order]
    starts = np.searchsorted(gs, np.arange(NC * NBLK))
    counts = np.diff(np.append(starts, node_ids.shape[0]))
    CPB = int(-(-counts.max() // P))
    rank = np.arange(node_ids.shape[0]) - starts[gs]
    j = rank >> 7
    p = rank & 127
    return order, c[order], blk[order], j, p, lane[order], int(CPB)


def prepare(E, edge_index, rev_index, W, b):
    """Host-side sharding. Returns (in_maps, meta)."""
    E = np.ascontiguousarray(E, dtype=np.float32)
    src = np.asarray(edge_index[0], dtype=np.int64)
    dest = np.asarray(edge_index[1], dtype=np.int64)
    rev = np.asarray(rev_index, dtype=np.int64)
    W = np.asarray(W, dtype=np.float32)
    b = np.asarray(b, dtype=np.float32)

    reluE = np.maximum(E, 0.0).astype(np.float16)

    o1, c1, blk1, j1, p1, lane1, CPB1 = _group_slots(dest)
    col1 = blk1 * CPB1 + j1
    o2, c2, blk2, j2, p2, lane2, CPB2 = _group_slots(src)
    col2 = blk2 * CPB2 + j2

    Wt_stack = np.ascontiguousarray(W.T.reshape(2, P, HID)).astype(np.float16)
    iota_row = np.ascontiguousarray(
        np.broadcast_to(np.arange(P, dtype=np.float32), (P, P))).astype(
        np.float16)
    iota_col = np.arange(P, dtype=np.float16).reshape(P, 1)

    in_maps = []
    metas = []
    for c in range(NC):
        m1 = c1 == c
        e1 = o1[m1]
        E_p1 = np.zeros((P, NBLK * CPB1, HID), np.float16)
        E_p1[p1[m1], col1[m1]] = reluE[e1]
        dest_lanes = np.full((P, NBLK * CPB1), PAD_LANE, np.float16)
        dest_lanes[p1[m1], col1[m1]] = lane1[m1].astype(np.float16)

        m2 = c2 == c
        e2 = o2[m2]
        nm = e2.shape[0]
        # ERV layout matches the dma_gather transpose output (block-r-major):
        #   ERV[q, blk*CPB2*256 + r*CPB2*128 + j*128 + i]
        #     = relu(E[rev[slot(blk, j, i)]])[r*128 + q]
        ERV = np.zeros((P, NBLK * CPB2 * HID), np.float16)
        rows = reluE[rev[e2]].reshape(nm, 2, P)
        jm, bm, im = j2[m2], blk2[m2], p2[m2]
        cbase = bm * CPB2 * HID + jm * P + im
        for r in range(2):
            ERV[:, cbase + r * (CPB2 * P)] = rows[:, r, :].T
        # gather indices: slot i of block bb lives at
        # src_idx[i % 16, bb*IPB + i // 16]; value = src node lane (pad -> 0)
        IPB = (CPB2 * P) // 16
        src_idx = np.zeros((16, NBLK * IPB), np.int16)
        slot = jm * P + im
        src_idx[slot % 16, bm * IPB + slot // 16] = lane2[m2].astype(np.int16)
        # the 16-partition wrapped index block must be replicated to all 8
        # GPSIMD cores (8 x 16 = 128 partitions)
        src_idx = np.tile(src_idx, (8, 1))

        in_maps.append({
            "E_p1": E_p1.reshape(P, NBLK * CPB1 * HID),
            "dest_lanes": dest_lanes,
            "ERV": ERV,
            "src_idx": src_idx,
            "Wt": Wt_stack,
            "iota_row": iota_row,
        })
        metas.append({"e2": e2, "p2": p2[m2], "col2": col2[m2]})

    meta = {"CPB1": CPB1, "CPB2": CPB2, "metas": metas, "b": b}
    return in_maps, meta


def build_program(CPB1, CPB2, reps=1):
    f32 = mybir.dt.float32
    f16 = mybir.dt.float16
    nc = bacc.Bacc("TRN2", target_bir_lowering=False, debug=False,
                   num_devices=NC)
    E_p1 = nc.dram_tensor("E_p1", [P, NBLK * CPB1 * HID], f16,
                          kind="ExternalInput").ap()
    dest_lanes = nc.dram_tensor("dest_lanes", [P, NBLK * CPB1], f16,
                                kind="ExternalInput").ap()
    ERV = nc.dram_tensor("ERV", [P, NBLK * CPB2 * HID], f16,
                         kind="ExternalInput").ap()
    IPB = (CPB2 * P) // 16
    src_idx = nc.dram_tensor("src_idx", [P, NBLK * IPB], mybir.dt.int16,
                             kind="ExternalInput").ap()
    Wt = nc.dram_tensor("Wt", [2, P, HID], f16, kind="ExternalInput").ap()
    iota_row = nc.dram_tensor("iota_row", [P, P], f16,
                              kind="ExternalInput").ap()
    out = nc.dram_tensor("out", [P, NBLK * CPB2 * HID], f16,
                         kind="ExternalOutput").ap()

    with tile.TileContext(nc) as tc:
        with ExitStack() as ctx:
            const = ctx.enter_context(tc.tile_pool(name="const", bufs=1))
            sb = ctx.enter_context(tc.tile_pool(name="sb", bufs=SB_BUFS))
            mvp = ctx.enter_context(tc.tile_pool(name="mv", bufs=1))
            ps_mv = ctx.enter_context(
                tc.tile_pool(name="ps_mv", bufs=PS_BUFS[0], space="PSUM"))
            ps_out = ctx.enter_context(
                tc.tile_pool(name="ps_out", bufs=PS_BUFS[1], space="PSUM"))

            wt0 = const.tile([P, HID], f16)
            nc.sync.dma_start(out=wt0[:], in_=Wt[0])
            wt1 = const.tile([P, HID], f16)
            nc.sync.dma_start(out=wt1[:], in_=Wt[1])
            iota_r = const.tile([P, P], f16)
            nc.sync.dma_start(out=iota_r[:], in_=iota_row[:])
            dest_t = const.tile([P, NBLK * CPB1], f16)
            nc.sync.dma_start(out=dest_t[:], in_=dest_lanes[:])
            src_idx_t = const.tile([P, NBLK * IPB], mybir.dt.int16)
            nc.sync.dma_start(out=src_idx_t[:], in_=src_idx[:])

            mv_all = mvp.tile([P, NBLK * HID], f16)  # resident M_v

            env = {
                "sb": sb, "mv_all": mv_all, "ps_mv": ps_mv,
                "ps_out": ps_out, "E_p1": E_p1, "ERV": ERV, "out": out,
                "dest_t": dest_t, "src_idx_t": src_idx_t, "IPB": IPB,
                "iota_r": iota_r, "wt0": wt0, "wt1": wt1,
            }
            for _rep in range(reps):
                _emit_body(nc, env, CPB1, CPB2)
    nc.compile()
    return nc


def _emit_p1(nc, env, CPB1, bb):
    f32 = mybir.dt.float32
    f16 = mybir.dt.float16
    sb = env["sb"]
    h_blk = sb.tile([P, CPB1 * HID], f16, tag="h_blk")
    if QUEUE_RR:
        h_eng = nc.scalar if bb % 2 == 0 else nc.sync
    else:
        h_eng = nc.sync if H_ENG == "sync" else nc.scalar
    h_eng.dma_start(
        out=h_blk[:],
        in_=env["E_p1"][:, bb * CPB1 * HID:(bb + 1) * CPB1 * HID])
    mv_ps = env["ps_mv"].tile([P, HID], f32, space="PSUM")
    if SKIP_P1MM:
        nc.vector.memset(env["mv_all"][:, bb * HID:(bb + 1) * HID], 0.25)
        return
    # all CPB1 one-hot S tiles in a single DVE op (fewer PE stalls)
    s_t = sb.tile([P, CPB1 * P], f16, tag="s_t")
    nc.vector.tensor_tensor(
        out=s_t[:].rearrange("p (j n) -> p j n", j=CPB1),
        in0=env["dest_t"][:, bb * CPB1:(bb + 1) * CPB1].rearrange(
            "p (j u) -> p j u", u=1).to_broadcast([P, CPB1, P]),
        in1=env["iota_r"][:].rearrange("p (u n) -> p u n", u=1).to_broadcast(
            [P, CPB1, P]),
        op=mybir.AluOpType.is_equal)
    for j in range(CPB1):
        nc.tensor.matmul(
            out=mv_ps[:], lhsT=s_t[:, j * P:(j + 1) * P],
            rhs=h_blk[:, j * HID:(j + 1) * HID],
            start=(j == 0), stop=(j == CPB1 - 1))
    if MV_COPY_ENG == "scalar":
        nc.scalar.activation(env["mv_all"][:, bb * HID:(bb + 1) * HID],
                             mv_ps[:], mybir.ActivationFunctionType.Copy)
    else:
        nc.vector.tensor_copy(out=env["mv_all"][:, bb * HID:(bb + 1) * HID],
                              in_=mv_ps[:])


def _emit_p2a(nc, env, CPB2, bb):
    f16 = mybir.dt.float16
    sb, mv_all = env["sb"], env["mv_all"]
    IPB = env["IPB"]
    NI = CPB2 * P  # slots (gather indices) per block
    erv = sb.tile([P, CPB2 * HID], f16, tag="erv")
    if QUEUE_RR:
        erv_eng = nc.sync if bb % 2 == 0 else nc.scalar
    else:
        erv_eng = nc.sync if ERV_ENG == "sync" else nc.scalar
    erv_eng.dma_start(
        out=erv[:], in_=env["ERV"][:, bb * CPB2 * HID:(bb + 1) * CPB2 * HID])
    # pv_T[q, r*NI + slot] = Mv[src_lane[slot], r*128+q], gathered straight
    # out of the block's resident Mv slice by the SWDGE transpose-gather.
    pv_sb = sb.tile([P, 2 * NI], f16, tag="pv_sb")
    if SKIP_GATHER:
        return erv, erv
    nc.gpsimd.dma_gather(
        pv_sb[:].rearrange("p (r i) -> p r i", r=2),
        mv_all[:, bb * HID:(bb + 1) * HID],
        env["src_idx_t"][:, bb * IPB:(bb + 1) * IPB],
        NI, NI, HID,
        transpose=True,
        sbuf_tokens_per_rank=P,
        sbuf_free_dim_per_rank=HID * 2,
    )
    return erv, pv_sb


def _emit_p2b(nc, env, CPB2, bb, erv, pv_sb):
    f32 = mybir.dt.float32
    f16 = mybir.dt.float16
    sb = env["sb"]
    NI = CPB2 * P
    muv = sb.tile([P, 2 * NI], f16, tag="muv")
    if SKIP_GATHER:
        muv = erv
    elif SPLIT_MUV:
        nc.vector.tensor_tensor(
            out=muv[:, 0:NI], in0=pv_sb[:, 0:NI], in1=erv[:, 0:NI],
            op=mybir.AluOpType.subtract)
        nc.vector.tensor_tensor(
            out=muv[:, NI:2 * NI], in0=pv_sb[:, NI:2 * NI],
            in1=erv[:, NI:2 * NI], op=mybir.AluOpType.subtract)
    else:
        nc.vector.tensor_tensor(
            out=muv[:], in0=pv_sb[:], in1=erv[:],
            op=mybir.AluOpType.subtract)
    out_blk = sb.tile([P, CPB2 * HID], f16, tag="out_blk")
    if SKIP_LIN:
        nc.vector.tensor_copy(out=out_blk[:], in_=muv[:])
        eng0 = nc.sync if OUT_ENG == "sync" else nc.scalar
        eng0.dma_start(
            out=env["out"][:, bb * CPB2 * HID:(bb + 1) * CPB2 * HID],
            in_=out_blk[:])
        return
    for og, w in _groups(CPB2):
        out_ps = env["ps_out"].tile([P, w * HID], f32, space="PSUM")
        for jj in range(w):
            j = og + jj
            nc.tensor.matmul(
                out=out_ps[:, jj * HID:(jj + 1) * HID],
                lhsT=muv[:, j * P:(j + 1) * P],
                rhs=env["wt0"][:], start=True, stop=False)
            nc.tensor.matmul(
                out=out_ps[:, jj * HID:(jj + 1) * HID],
                lhsT=muv[:, NI + j * P:NI + (j + 1) * P],
                rhs=env["wt1"][:], start=False, stop=True)
        if OUT_COPY_ENG == "scalar":
            nc.scalar.activation(out_blk[:, og * HID:(og + w) * HID],
                                 out_ps[:], mybir.ActivationFunctionType.Copy)
        else:
            nc.vector.tensor_copy(out=out_blk[:, og * HID:(og + w) * HID],
                                  in_=out_ps[:])
    # alternate the output-write queue to balance the two HWDGE rings
    if QUEUE_RR:
        eng = nc.sync if bb % 2 == 0 else nc.scalar
    elif OUT_DMA_ALT:
        eng = nc.sync if bb % 2 == 0 else nc.scalar
    else:
        eng = nc.sync if OUT_ENG == "sync" else nc.scalar
    eng.dma_start(
        out=env["out"][:, bb * CPB2 * HID:(bb + 1) * CPB2 * HID],
        in_=out_blk[:])


def _emit_body(nc, env, CPB1, CPB2):
    # software pipeline: the gather of block bb (p2a) is issued right after
    # p1(bb); the consume stage (p2b) trails by one block, so the SWDGE
    # gather + subtract latency hides behind p1(bb+1)'s PE work.
    pend = {}
    for bb in range(NBLK):
        _emit_p1(nc, env, CPB1, bb)
        pend[bb] = _emit_p2a(nc, env, CPB2, bb)
        if bb - 1 in pend:
            _emit_p2b(nc, env, CPB2, bb - 1, *pend.pop(bb - 1))
    _emit_p2b(nc, env, CPB2, NBLK - 1, *pend.pop(NBLK - 1))


def assemble(results, meta):
    CPB2 = meta["CPB2"]
    b = meta["b"]
    out_full = np.empty((N_EDGES, HID), np.float32)
    for c in range(NC):
        mc = meta["metas"][c]
        arr = results[c]["out"].reshape(P, NBLK * CPB2, HID)
        out_full[mc["e2"]] = arr[mc["p2"], mc["col2"]]
    out_full += b
    return out_full


def kernel(E, edge_index, rev_index, W, b):
    in_maps, meta = prepare(E, edge_index, rev_index, W, b)
    nc = build_program(meta["CPB1"], meta["CPB2"])
    res = run_bass_kernel_spmd(nc, in_maps, list(range(NC)))
    return assemble(res.results, meta)


# revision 29
# speedup vs baseline: 3.5957x; 3.5957x over previous
"""Trainium2 Bass kernel for a Chemprop GNN message-passing layer.

Reference computation (single layer, n_nodes=50000, n_edges=300000, hidden=256):
    H   = relu(E)                                  # [E, 256]
    M_v = segment_sum(H, dest, n_nodes)            # [V, 256]
    out = (M_v[src] - H[rev]) @ W.T + b            # [E, 256]

Distribution over 8 NeuronCores (zero collectives): nodes are sharded; core c
owns node range [c*6250, (c+1)*6250), as 49 blocks of 128 node lanes.

Host prep (pure permutation / gather / relu, all O(E*H)):
  * Phase 1 (segment sum): edges grouped by dest-node block; relu(E) rows
    written to a [128, NBLK*CPB1*256] f16 slab in (lane, block, chunk, hid)
    order so each block's DMA is contiguous per partition.
  * Phase 2: edges grouped by src-node block. relu(E)[rev] rows are
    PRE-GATHERED on host and stored TRANSPOSED per chunk:
    ERV[p, (blk*CPB2+j)*256 + r*128 + i] = relu(E[rev[slot(blk,j,i)]])[r*128+p].
    This removes all device-side indirect gathers and PE transposes.

Device per block bb (software-pipelined: phase1(bb+1) emitted before
phase2(bb) so PE never stalls on the Mv PSUM->SBUF copy):
  phase 1: stream h_blk; per chunk build one-hot S[e,n]=(dest_lane[e]==n) on
    DVE and accumulate Mv_ps += S.T @ H_chunk on PE; ScalarE copies Mv to a
    resident f16 SBUF tile (49 x [128,256]).
  phase 2: stream erv (already relu'd, transposed, negatable); build
    R[n,e]=(src_lane[e]==n) from a partition_broadcast (GPSIMD) of the src
    lane row; per chunk:
      pv_T[r*128+q, i] = Mv[:, r-half].T @ R  (2 matmuls into one PSUM tile)
      muv_T = pv_T - erv_chunk                (DVE, f16 out)
      out_ps = muv_T[0:128].T @ W.T[0:128] + muv_T[128:256].T @ W.T[128:256]
      ScalarE copies out_ps (f32) -> out_blk (f16)
    One DMA writes the block's outputs; bias is added on host in assemble().
"""

import sys
from contextlib import ExitStack

import numpy as np

sys.path.insert(0, "/opt/trn_rl_repo")

import concourse.bass as bass
import concourse.bacc as bacc
import concourse.tile as tile
from concourse import mybir
from concourse.bass_utils import run_bass_kernel_spmd

N_NODES = 50000
N_EDGES = 300000
HID = 256
NC = 8
P = 128
NPC = N_NODES // NC          # 6250 nodes per core
NBLK = (NPC + P - 1) // P    # 49 blocks of 128 node lanes per core
PAD_LANE = 200.0             # sentinel lane value -> one-hot row of zeros
SB_BUFS = 4
PS_BUFS = (2, 3, 3)          # psum bufs: mv, pv, out
OUT_COPY_ENG = "scalar"      # engine for PSUM->SBUF out copies
MV_COPY_ENG = "scalar"       # engine for PSUM->SBUF Mv copies
OUT_DMA_ALT = False          # alternate out-write queue between SP/Act
H_ENG = "scalar"             # queue for h_blk loads
ERV_ENG = "sync"             # queue for erv loads
OUT_ENG = "sync"             # queue for out writes (when not alternating)
SPLIT_MUV = False            # subtract muv halves separately
SKIP_P1MM = False            # ablation: skip phase-1 s_t + matmuls
SKIP_LIN = False             # ablation: skip linear matmuls + out copies
SKIP_GATHER = False          # ablation: skip gather+subtract (muv <- erv)
QUEUE_RR = False             # round-robin h/erv/out across both HWDGE queues


def _groups(cpb):
    """Split cpb chunks into pairs (last group may be a single)."""
    gs = []
    j = 0
    while j < cpb:
        w = min(2, cpb - j)
        gs.append((j, w))
        j += w
    return gs


def _group_slots(node_ids):
    """Group edges by (core, block) of node ownership; assign (chunk, lane)
    slots. Returns (order, core, blk, j, p, lane, CPB)."""
    c = node_ids // NPC
    loc = node_ids - c * NPC
    blk = loc >> 7
    lane = loc & 127
    g = c * NBLK + blk
    order = np.argsort(g, kind="stable")
    gs = g[order]
    starts = np.searchsorted(gs, np.arange(NC * NBLK))
    counts = np.diff(np.append(starts, node_ids.shape[0]))
    CPB = int(-(-counts.max() // P))
    rank = np.arange(node_ids.shape[0]) - starts[gs]
    j = rank >> 7
    p = rank & 127
    return order, c[order], blk[order], j, p, lane[order], int(CPB)


def prepare(E, edge_index, rev_index, W, b):
    """Host-side sharding. Returns (in_maps, meta)."""
    E = np.ascontiguousarray(E, dtype=np.float32)
    src = np.asarray(edge_index[0], dtype=np.int64)
    dest = np.asarray(edge_index[1], dtype=np.int64)
    rev = np.asarray(rev_index, dtype=np.int64)
    W = np.asarray(W, dtype=np.float32)
    b = np.asarray(b, dtype=np.float32)

    reluE = np.maximum(E, 0.0).astype(np.float16)

    o1, c1, blk1, j1, p1, lane1, CPB1 = _group_slots(dest)
    col1 = blk1 * CPB1 + j1
    o2, c2, blk2, j2, p2, lane2, CPB2 = _group_slots(src)
    col2 = blk2 * CPB2 + j2

    Wt_stack = np.ascontiguousarray(W.T.reshape(2, P, HID)).astype(np.float16)
    iota_row = np.ascontiguousarray(
        np.broadcast_to(np.arange(P, dtype=np.float32), (P, P))).astype(
        np.float16)
    iota_col = np.arange(P, dtype=np.float16).reshape(P, 1)

    in_maps = []
    metas = []
    for c in range(NC):
        m1 = c1 == c
        e1 = o1[m1]
        E_p1 = np.zeros((P, NBLK * CPB1, HID), np.float16)
        E_p1[p1[m1], col1[m1]] = reluE[e1]
        dest_lanes = np.full((P, NBLK * CPB1), PAD_LANE, np.float16)
        dest_lanes[p1[m1], col1[m1]] = lane1[m1].astype(np.float16)

        m2 = c2 == c
        e2 = o2[m2]
        nm = e2.shape[0]
        # ERV layout matches the dma_gather transpose output (block-r-major):
        #   ERV[q, blk*CPB2*256 + r*CPB2*128 + j*128 + i]
        #     = relu(E[rev[slot(blk, j, i)]])[r*128 + q]
        ERV = np.zeros((P, NBLK * CPB2 * HID), np.float16)
        rows = reluE[rev[e2]].reshape(nm, 2, P)
        jm, bm, im = j2[m2], blk2[m2], p2[m2]
        cbase = bm * CPB2 * HID + jm * P + im
        for r in range(2):
            ERV[:, cbase + r * (CPB2 * P)] = rows[:, r, :].T
        src_row = np.full((1, NBLK * CPB2 * P), PAD_LANE, np.float16)
        src_row[0, col2[m2] * P + p2[m2]] = lane2[m2].astype(np.float16)

        in_maps.append({
            "E_p1": E_p1.reshape(P, NBLK * CPB1 * HID),
            "dest_lanes": dest_lanes,
            "ERV": ERV,
            "src_row": src_row,
            "Wt": Wt_stack,
            "iota_row": iota_row,
            "iota_col": iota_col,
        })
        metas.append({"e2": e2, "p2": p2[m2], "col2": col2[m2]})

    meta = {"CPB1": CPB1, "CPB2": CPB2, "metas": metas, "b": b}
    return in_maps, meta


def build_program(CPB1, CPB2, reps=1):
    f32 = mybir.dt.float32
    f16 = mybir.dt.float16
    nc = bacc.Bacc("TRN2", target_bir_lowering=False, debug=False,
                   num_devices=NC)
    E_p1 = nc.dram_tensor("E_p1", [P, NBLK * CPB1 * HID], f16,
                          kind="ExternalInput").ap()
    dest_lanes = nc.dram_tensor("dest_lanes", [P, NBLK * CPB1], f16,
                                kind="ExternalInput").ap()
    ERV = nc.dram_tensor("ERV", [P, NBLK * CPB2 * HID], f16,
                         kind="ExternalInput").ap()
    src_row = nc.dram_tensor("src_row", [1, NBLK * CPB2 * P], f16,
                             kind="ExternalInput").ap()
    iota_col = nc.dram_tensor("iota_col", [P, 1], f16,
                              kind="ExternalInput").ap()
    Wt = nc.dram_tensor("Wt", [2, P, HID], f16, kind="ExternalInput").ap()
    iota_row = nc.dram_tensor("iota_row", [P, P], f16,
                              kind="ExternalInput").ap()
    out = nc.dram_tensor("out", [P, NBLK * CPB2 * HID], f16,
                         kind="ExternalOutput").ap()

    with tile.TileContext(nc) as tc:
        with ExitStack() as ctx:
            const = ctx.enter_context(tc.tile_pool(name="const", bufs=1))
            sb = ctx.enter_context(tc.tile_pool(name="sb", bufs=SB_BUFS))
            mvp = ctx.enter_context(tc.tile_pool(name="mv", bufs=1))
            ps_mv = ctx.enter_context(
                tc.tile_pool(name="ps_mv", bufs=PS_BUFS[0], space="PSUM"))
            ps_pv = ctx.enter_context(
                tc.tile_pool(name="ps_pv", bufs=PS_BUFS[1], space="PSUM"))
            ps_out = ctx.enter_context(
                tc.tile_pool(name="ps_out", bufs=PS_BUFS[2], space="PSUM"))

            wt0 = const.tile([P, HID], f16)
            nc.sync.dma_start(out=wt0[:], in_=Wt[0])
            wt1 = const.tile([P, HID], f16)
            nc.sync.dma_start(out=wt1[:], in_=Wt[1])
            iota_r = const.tile([P, P], f16)
            nc.sync.dma_start(out=iota_r[:], in_=iota_row[:])
            dest_t = const.tile([P, NBLK * CPB1], f16)
            nc.sync.dma_start(out=dest_t[:], in_=dest_lanes[:])
            src_sb = const.tile([1, NBLK * CPB2 * P], f16)
            nc.sync.dma_start(out=src_sb[:], in_=src_row[:])
            iota_c = const.tile([P, 1], f16)
            nc.sync.dma_start(out=iota_c[:], in_=iota_col[:])

            mv_all = mvp.tile([P, NBLK * HID], f16)  # resident M_v

            env = {
                "sb": sb, "mv_all": mv_all, "ps_mv": ps_mv, "ps_pv": ps_pv,
                "ps_out": ps_out, "E_p1": E_p1, "ERV": ERV, "out": out,
                "dest_t": dest_t, "src_sb": src_sb, "iota_c": iota_c,
                "iota_r": iota_r, "wt0": wt0, "wt1": wt1,
            }
            for _rep in range(reps):
                _emit_body(nc, env, CPB1, CPB2)
    nc.compile()
    return nc


def _emit_p1(nc, env, CPB1, bb):
    f32 = mybir.dt.float32
    f16 = mybir.dt.float16
    sb = env["sb"]
    h_blk = sb.tile([P, CPB1 * HID], f16, tag="h_blk")
    if QUEUE_RR:
        h_eng = nc.scalar if bb % 2 == 0 else nc.sync
    else:
        h_eng = nc.sync if H_ENG == "sync" else nc.scalar
    h_eng.dma_start(
        out=h_blk[:],
        in_=env["E_p1"][:, bb * CPB1 * HID:(bb + 1) * CPB1 * HID])
    mv_ps = env["ps_mv"].tile([P, HID], f32, space="PSUM")
    if SKIP_P1MM:
        nc.vector.memset(env["mv_all"][:, bb * HID:(bb + 1) * HID], 0.25)
        return
    # all CPB1 one-hot S tiles in a single DVE op (fewer PE stalls)
    s_t = sb.tile([P, CPB1 * P], f16, tag="s_t")
    nc.vector.tensor_tensor(
        out=s_t[:].rearrange("p (j n) -> p j n", j=CPB1),
        in0=env["dest_t"][:, bb * CPB1:(bb + 1) * CPB1].rearrange(
            "p (j u) -> p j u", u=1).to_broadcast([P, CPB1, P]),
        in1=env["iota_r"][:].rearrange("p (u n) -> p u n", u=1).to_broadcast(
            [P, CPB1, P]),
        op=mybir.AluOpType.is_equal)
    for j in range(CPB1):
        nc.tensor.matmul(
            out=mv_ps[:], lhsT=s_t[:, j * P:(j + 1) * P],
            rhs=h_blk[:, j * HID:(j + 1) * HID],
            start=(j == 0), stop=(j == CPB1 - 1))
    if MV_COPY_ENG == "scalar":
        nc.scalar.activation(env["mv_all"][:, bb * HID:(bb + 1) * HID],
                             mv_ps[:], mybir.ActivationFunctionType.Copy)
    else:
        nc.vector.tensor_copy(out=env["mv_all"][:, bb * HID:(bb + 1) * HID],
                              in_=mv_ps[:])


def _emit_p2a(nc, env, CPB2, bb):
    f16 = mybir.dt.float16
    sb, mv_all = env["sb"], env["mv_all"]
    NI = CPB2 * P  # slots (gather indices) per block
    erv = sb.tile([P, CPB2 * HID], f16, tag="erv")
    if QUEUE_RR:
        erv_eng = nc.sync if bb % 2 == 0 else nc.scalar
    else:
        erv_eng = nc.sync if ERV_ENG == "sync" else nc.scalar
    erv_eng.dma_start(
        out=erv[:], in_=env["ERV"][:, bb * CPB2 * HID:(bb + 1) * CPB2 * HID])
    # one-hot R[n, slot] = (src_lane[slot] == n), built from a
    # partition-broadcast of the src lane row (GPSIMD) + is_equal (DVE)
    rb_bc = sb.tile([P, NI], f16, tag="rb_bc")
    nc.gpsimd.partition_broadcast(
        rb_bc[:], env["src_sb"][0:1, bb * NI:(bb + 1) * NI])
    rb = sb.tile([P, NI], f16, tag="rb")
    nc.vector.tensor_tensor(
        out=rb[:], in0=rb_bc[:],
        in1=env["iota_c"][:, 0:1].to_broadcast([P, NI]),
        op=mybir.AluOpType.is_equal)
    return erv, rb


def _emit_p2b(nc, env, CPB2, bb, erv, rb):
    f32 = mybir.dt.float32
    f16 = mybir.dt.float16
    sb, mv_all = env["sb"], env["mv_all"]
    NI = CPB2 * P
    erv3 = erv[:].rearrange("p (r i) -> p r i", r=2)
    out_blk = sb.tile([P, CPB2 * HID], f16, tag="out_blk")
    for og, w in _groups(CPB2):
        # paired pv: one matmul per Mv half covers w chunks
        pv_ps = env["ps_pv"].tile([P, w * HID], f32, space="PSUM")
        nc.tensor.matmul(
            out=pv_ps[:, 0:w * P], lhsT=mv_all[:, bb * HID:bb * HID + P],
            rhs=rb[:, og * P:(og + w) * P], start=True, stop=True)
        nc.tensor.matmul(
            out=pv_ps[:, w * P:2 * w * P],
            lhsT=mv_all[:, bb * HID + P:(bb + 1) * HID],
            rhs=rb[:, og * P:(og + w) * P], start=True, stop=True)
        muv = sb.tile([P, w * HID], f16, tag="muv")
        nc.vector.tensor_tensor(
            out=muv[:].rearrange("p (r i) -> p r i", r=2),
            in0=pv_ps[:].rearrange("p (r i) -> p r i", r=2),
            in1=erv3[:, :, og * P:(og + w) * P],
            op=mybir.AluOpType.subtract)
        out_ps = env["ps_out"].tile([P, w * HID], f32, space="PSUM")
        for jj in range(w):
            if SKIP_LIN:
                continue
            nc.tensor.matmul(
                out=out_ps[:, jj * HID:(jj + 1) * HID],
                lhsT=muv[:, jj * P:(jj + 1) * P],
                rhs=env["wt0"][:], start=True, stop=False)
            nc.tensor.matmul(
                out=out_ps[:, jj * HID:(jj + 1) * HID],
                lhsT=muv[:, w * P + jj * P:w * P + (jj + 1) * P],
                rhs=env["wt1"][:], start=False, stop=True)
        if OUT_COPY_ENG == "scalar":
            nc.scalar.activation(out_blk[:, og * HID:(og + w) * HID],
                                 out_ps[:], mybir.ActivationFunctionType.Copy)
        else:
            nc.vector.tensor_copy(out=out_blk[:, og * HID:(og + w) * HID],
                                  in_=out_ps[:])
    # output-write queue
    if QUEUE_RR:
        eng = nc.sync if bb % 2 == 0 else nc.scalar
    elif OUT_DMA_ALT:
        eng = nc.sync if bb % 2 == 0 else nc.scalar
    else:
        eng = nc.sync if OUT_ENG == "sync" else nc.scalar
    eng.dma_start(
        out=env["out"][:, bb * CPB2 * HID:(bb + 1) * CPB2 * HID],
        in_=out_blk[:])


def _emit_body(nc, env, CPB1, CPB2):
    # software pipeline: the gather of block bb (p2a) is issued right after
    # p1(bb); the consume stage (p2b) trails by one block, so the SWDGE
    # gather + subtract latency hides behind p1(bb+1)'s PE work.
    pend = {}
    for bb in range(NBLK):
        _emit_p1(nc, env, CPB1, bb)
        pend[bb] = _emit_p2a(nc, env, CPB2, bb)
        if bb - 1 in pend:
            _emit_p2b(nc, env, CPB2, bb - 1, *pend.pop(bb - 1))
    _emit_p2b(nc, env, CPB2, NBLK - 1, *pend.pop(NBLK - 1))


def assemble(results, meta):
    CPB2 = meta["CPB2"]
    b = meta["b"]
    out_full = np.empty((N_EDGES, HID), np.float32)
    for c in range(NC):
        mc = meta["metas"][c]
        arr = results[c]["out"].reshape(P, NBLK * CPB2, HID)
        out_full[mc["e2"]] = arr[mc["p2"], mc["col2"]]
    out_full += b
    return out_full


def kernel(E, edge_index, rev_index, W, b):
    in_maps, meta = prepare(E, edge_index, rev_index, W, b)
    nc = build_program(meta["CPB1"], meta["CPB2"])
    res = run_bass_kernel_spmd(nc, in_maps, list(range(NC)))
    return assemble(res.results, meta)
